# revision 3
# baseline (speedup 1.0000x reference)
"""Fused multi-head-free attention kernel for Trainium2 (Bass/Tile), 8-core SPMD.

Problem: nn_Attention — per batch element b:
    q = query[b] @ Wq + bq          [Sq, H]
    k = key[b]   @ Wk + bk          [Skv, H]
    v = value[b] @ Wv + bv          [Skv, H]
    S = q @ k.T                     [Sq, Skv]
    W = softmax(S, axis=-1)
    C = W @ v                       [Sq, H]
    returns (C, W)

Sharding: pure data-parallel over batch (B=8 == 8 cores), one batch element
per NeuronCore; projection weights replicated. No collectives.

Per-core dataflow (all matmuls on PE, fp32r for the numerically sensitive
path, bf16 for the post-softmax context path):
  1. x in {key, query, value} is DMA'd in 128-row tiles and transposed on PE
     (fp32r transpose via identity) into xT [D, S] layout in SBUF.
  2. qT/kT [H, S] = Wt.T @ xT (fp32r), bias added per-partition during the
     PSUM->SBUF copy.  v is computed in natural [Skv, H] layout (bf16 out).
  3. Per 128-row q-tile: S tile [128, Skv] = qT.T @ kT into 4 PSUM banks,
     row-max on DVE, exp(S - max) on ACT (with per-bank row-sum accumulation),
     normalize -> W (fp32, DMA'd out), and a bf16 copy of W is PE-transposed
     to feed C = W @ v accumulation (bf16), C copied out in fp32.
"""

import numpy as np

B, SQ, SKV, D, H = 8, 2048, 2048, 512, 512
P = 128                 # partitions
ST = SQ // P            # 16 s-tiles
DC = D // P             # 4 contraction chunks
HC = H // P             # 4 h tiles
NB = SKV // 512         # 4 psum banks per score row

_CACHE = {}


def _patch_multiwaits(nc, mb):
    """walrus in this container rejects >1 sync-wait per instruction
    (setupSyncWait: "Too many sync wait commands").  Split extra waits onto
    preceding same-engine NoOps — engine streams are in-order so semantics
    are preserved."""
    for blk in nc.m.functions[0].blocks:
        insts = list(blk.instructions)
        new_insts, changed = [], False
        for inst in insts:
            si = getattr(inst, "sync_info", None)
            if si is not None and si.on_wait and len(si.on_wait) > 1:
                waits = list(si.on_wait)
                extra, keep = waits[:-1], waits[-1:]
                for k, w in enumerate(extra):
                    new_insts.append(mb.InstNoOp(
                        name=f"{inst.name}-ws{k}",
                        sync_info=mb.SyncInfo(on_wait=[w], on_update=[]),
                        bass_nofuse=True, engine=inst.engine))
                si.on_wait = keep
                changed = True
            new_insts.append(inst)
        if changed:
            blk.instructions = new_insts


def _build():
    import concourse.bass as bass
    import concourse.tile as tile
    from concourse import mybir as mb

    F32, F32R, BF16 = mb.dt.float32, mb.dt.float32r, mb.dt.bfloat16
    AX = mb.AxisListType.X
    EXP = mb.ActivationFunctionType.Exp

    nc = bass.Bass("TRN2", target_bir_lowering=False, debug=False, num_devices=1)

    dq = nc.dram_tensor("query", (SQ, D), F32, kind="ExternalInput").ap()
    dk = nc.dram_tensor("key", (SKV, D), F32, kind="ExternalInput").ap()
    dv = nc.dram_tensor("value", (SKV, D), F32, kind="ExternalInput").ap()
    dW = {t: nc.dram_tensor(f"W{t}", (D, H), F32, kind="ExternalInput").ap()
          for t in "qkv"}
    db = {t: nc.dram_tensor(f"b{t}", (H,), F32, kind="ExternalInput").ap()
          for t in "qkv"}
    dident = nc.dram_tensor("ident", (P, P), F32, kind="ExternalInput").ap()
    dones = nc.dram_tensor("ones", (1, 1), F32, kind="ExternalInput").ap()
    dctx = nc.dram_tensor("context", (SQ, H), F32, kind="ExternalOutput").ap()
    dwei = nc.dram_tensor("weights", (SQ, SKV), F32, kind="ExternalOutput").ap()

    with tile.TileContext(nc) as tc:
        with tc.tile_pool(name="const", bufs=1) as const, \
             tc.tile_pool(name="big", bufs=1) as big, \
             tc.tile_pool(name="smal", bufs=2) as smal:

            ident_r = const.tile([P, P], F32R)
            nc.sync.dma_start(out=ident_r, in_=dident.bitcast(F32R))
            ident_b = const.tile([P, P], BF16)
            nc.vector.tensor_copy(ident_b, ident_r.bitcast(F32))
            ones_r = const.tile([1, 1], F32)
            nc.sync.dma_start(out=ones_r, in_=dones)
            # bv broadcast to all partitions (step-0 partition DMA)
            bvb = const.tile([P, H], F32)
            bv_bcast = bass.AP(tensor=db["v"].tensor, offset=db["v"].offset,
                               ap=[[0, P]] + db["v"].ap)
            nc.sync.dma_start(out=bvb, in_=bv_bcast)

            # persistent activations
            qT = [big.tile([P, SQ], F32R, tag=f"qt{h}", name=f"qt{h}") for h in range(HC)]
            kT = [big.tile([P, SKV], F32R, tag=f"kt{h}", name=f"kt{h}") for h in range(HC)]
            vb = big.tile([P, ST, 512], BF16, tag="vb")

            # ---------------- phase 0/1: weights, biases, inputs ----------
            with tc.tile_pool(name="wpool", bufs=1) as wpool, \
                 tc.tile_pool(name="xstage", bufs=4) as xstage, \
                 tc.tile_pool(name="xt", bufs=1) as xtp, \
                 tc.tile_pool(name="ps1", bufs=2, space="PSUM") as ps1, \
                 tc.tile_pool(name="ps1b", bufs=2, space="PSUM") as ps1b:

                w = {}
                for t in "qkv":
                    w[t] = wpool.tile([P, DC, H], F32R, tag=f"w{t}", name=f"w{t}")
                    nc.sync.dma_start(
                        out=w[t],
                        in_=dW[t].rearrange("(c p) h -> p c h", p=P).bitcast(F32R))
                brow = {}
                for t in "qk":
                    brow[t] = const.tile([1, H], F32, name=f"brow{t}")
                    nc.sync.dma_start(out=brow[t], in_=db[t][None, :])
                # transpose bq/bk to per-partition layout via K=1 matmuls
                bT = {}
                for t in "qk":
                    bT[t] = const.tile([P, HC], F32, name=f"bT{t}")
                    for h in range(HC):
                        pb = ps1b.tile([P, 1], F32, tag="bias")
                        nc.tensor.matmul(pb, brow[t][:, h * P:(h + 1) * P],
                                         ones_r, start=True, stop=True)
                        nc.vector.tensor_copy(bT[t][:, h:h + 1], pb)

                xt = [xtp.tile([P, SQ], F32R, tag=f"xt{c}", name=f"xt{c}") for c in range(DC)]

                def load_transpose(dram_t):
                    for i in range(ST):
                        xs = xstage.tile([P, D], F32R, tag="xs")
                        nc.sync.dma_start(
                            out=xs, in_=dram_t[i * P:(i + 1) * P, :].bitcast(F32R))
                        for c in range(DC):
                            pt = ps1.tile([P, P], F32R, tag="tp")
                            nc.tensor.transpose(pt, xs[:, c * P:(c + 1) * P], ident_r)
                            nc.vector.tensor_copy(xt[c][:, i * P:(i + 1) * P], pt)

                def project_T(t, dst):        # dst[h][:, s] = (x@Wt+bt).T
                    for h in range(HC):
                        for cc in range(DC):
                            pp = ps1.tile([P, 512], F32, tag="proj")
                            for c in range(DC):
                                nc.tensor.matmul(
                                    pp, w[t][:, c, h * P:(h + 1) * P],
                                    xt[c][:, cc * 512:(cc + 1) * 512],
                                    start=(c == 0), stop=(c == DC - 1))
                            nc.vector.tensor_scalar_add(
                                dst[h][:, cc * 512:(cc + 1) * 512], pp,
                                bT[t][:, h:h + 1])

                load_transpose(dk)
                project_T("k", kT)
                load_transpose(dq)
                project_T("q", qT)
                load_transpose(dv)
                for j in range(ST):           # v natural [kv, H] in bf16
                    pp = ps1.tile([P, 512], F32, tag="proj")
                    for c in range(DC):
                        nc.tensor.matmul(pp, xt[c][:, j * P:(j + 1) * P],
                                         w["v"][:, c, :],
                                         start=(c == 0), stop=(c == DC - 1))
                    nc.vector.tensor_tensor(out=vb[:, j, :], in0=pp, in1=bvb,
                                            op=mb.AluOpType.add)

            # ---------------- phase 2: attention per q-tile ----------------
            with tc.tile_pool(name="upool", bufs=2) as upool, \
                 tc.tile_pool(name="wout", bufs=2) as wout, \
                 tc.tile_pool(name="wtbs", bufs=8) as wtbs, \
                 tc.tile_pool(name="pssc", bufs=4, space="PSUM") as pssc, \
                 tc.tile_pool(name="pstb", bufs=2, space="PSUM") as pstb, \
                 tc.tile_pool(name="psctx", bufs=2, space="PSUM") as psctx:

                for i in range(ST):
                    qs = slice(i * P, (i + 1) * P)
                    # scores: 4 banks of [128, 512]
                    sc = []
                    nmax4 = smal.tile([P, NB], F32, tag="nmax4")
                    for cc in range(NB):
                        s_ = pssc.tile([P, 512], F32, tag="sc")
                        for h in range(HC):
                            nc.tensor.matmul(s_, qT[h][:, qs],
                                             kT[h][:, cc * 512:(cc + 1) * 512],
                                             start=(h == 0), stop=(h == HC - 1))
                        nc.vector.reduce_max(nmax4[:, cc:cc + 1], s_, axis=AX)
                        sc.append(s_)
                    nm = smal.tile([P, 1], F32, tag="nm")
                    nc.vector.reduce_max(nm, nmax4, axis=AX, negate=True)

                    U = upool.tile([P, SKV], F32, tag="U")
                    sums4 = smal.tile([P, NB], F32, tag="sums4")
                    for cc in range(NB):
                        nc.scalar.activation(U[:, cc * 512:(cc + 1) * 512], sc[cc],
                                             EXP, bias=nm, scale=1.0,
                                             accum_out=sums4[:, cc:cc + 1])
                    ssum = smal.tile([P, 1], F32, tag="ssum")
                    nc.vector.reduce_sum(ssum, sums4, axis=AX)
                    rc = smal.tile([P, 1], F32, tag="rc")
                    nc.vector.reciprocal(rc, ssum)

                    # W fp32 out (normalize on GPSIMD, DMA to DRAM)
                    Wt_ = wout.tile([P, SKV], F32, tag="W")
                    nc.gpsimd.tensor_scalar_mul(Wt_, U, rc)
                    nc.sync.dma_start(out=dwei[qs, :], in_=Wt_)

                    # bf16 normalized W -> PE transpose -> context accum
                    Wb = wout.tile([P, SKV], BF16, tag="Wb")
                    nc.vector.tensor_scalar_mul(Wb, U, rc)
                    pc = psctx.tile([P, 512], F32, tag="ctx")
                    for g in range(4):
                        pt = pstb.tile([P, 512], BF16, tag="wtb")
                        for jj in range(4):
                            j = g * 4 + jj
                            nc.tensor.transpose(pt[:, jj * P:(jj + 1) * P],
                                                Wb[:, j * P:(j + 1) * P], ident_b)
                        wt_s = wtbs.tile([P, 512], BF16, tag="wtbs")
                        nc.vector.tensor_copy(wt_s, pt)
                        for jj in range(4):
                            j = g * 4 + jj
                            nc.tensor.matmul(pc, wt_s[:, jj * P:(jj + 1) * P],
                                             vb[:, j, :],
                                             start=(j == 0), stop=(j == ST - 1))
                    Ct = smal.tile([P, 512], F32, tag="C")
                    nc.vector.tensor_copy(Ct, pc)
                    nc.sync.dma_start(out=dctx[qs, :], in_=Ct)

    _patch_multiwaits(nc, mb)
    return nc


def kernel(**inputs):
    from concourse.bass_utils import run_bass_kernel_spmd

    if "nc" not in _CACHE:
        _CACHE["nc"] = _build()
    nc = _CACHE["nc"]

    query = np.asarray(inputs["query"], dtype=np.float32)
    key = np.asarray(inputs["key"], dtype=np.float32)
    value = np.asarray(inputs["value"], dtype=np.float32)
    consts = {
        "Wq": np.asarray(inputs["Wq"], np.float32),
        "Wk": np.asarray(inputs["Wk"], np.float32),
        "Wv": np.asarray(inputs["Wv"], np.float32),
        "bq": np.asarray(inputs["bq"], np.float32),
        "bk": np.asarray(inputs["bk"], np.float32),
        "bv": np.asarray(inputs["bv"], np.float32),
        "ident": np.eye(P, dtype=np.float32),
        "ones": np.ones((1, 1), np.float32),
    }
    in_maps = [dict(consts, query=query[b], key=key[b], value=value[b])
               for b in range(B)]
    res = run_bass_kernel_spmd(nc, in_maps, core_ids=list(range(B)),
                               **_CACHE.get("run_kwargs", {}))
    _CACHE["last_results"] = res
    context = np.stack([res.results[b]["context"] for b in range(B)])
    weights = np.stack([res.results[b]["weights"] for b in range(B)])
    return (context, weights)


# revision 6
# speedup vs baseline: 1.9769x; 1.9769x over previous
"""Fused multi-head-free attention kernel for Trainium2 (Bass/Tile), 8-core SPMD.

Problem: nn_Attention — per batch element b:
    q = query[b] @ Wq + bq          [Sq, H]
    k = key[b]   @ Wk + bk          [Skv, H]
    v = value[b] @ Wv + bv          [Skv, H]
    S = q @ k.T                     [Sq, Skv]
    W = softmax(S, axis=-1)
    C = W @ v                       [Sq, H]
    returns (C, W)

Sharding: pure data-parallel over batch (B=8 == 8 cores), one batch element
per NeuronCore; projection weights replicated. No collectives.

Per-core dataflow (all matmuls on PE, fp32r for the numerically sensitive
path, bf16 for the post-softmax context path):
  1. x in {key, query, value} is DMA'd in 128-row tiles and transposed on PE
     (fp32r transpose via identity) into xT [D, S] layout in SBUF.
  2. qT/kT [H, S] = Wt.T @ xT (fp32r), bias added per-partition during the
     PSUM->SBUF copy.  v is computed in natural [Skv, H] layout (bf16 out).
  3. Per 128-row q-tile: S tile [128, Skv] = qT.T @ kT into 4 PSUM banks,
     row-max on DVE, exp(S - max) on ACT (with per-bank row-sum accumulation),
     normalize -> W (fp32, DMA'd out), and a bf16 copy of W is PE-transposed
     to feed C = W @ v accumulation (bf16), C copied out in fp32.
"""

import numpy as np

B, SQ, SKV, D, H = 8, 2048, 2048, 512, 512
P = 128                 # partitions
ST = SQ // P            # 16 s-tiles
DC = D // P             # 4 contraction chunks
HC = H // P             # 4 h tiles
NB = SKV // 512         # 4 psum banks per score row

_CACHE = {}


def _patch_multiwaits(nc, mb):
    """walrus in this container rejects >1 sync-wait per instruction
    (setupSyncWait: "Too many sync wait commands").  Split extra waits onto
    preceding same-engine NoOps — engine streams are in-order so semantics
    are preserved."""
    for blk in nc.m.functions[0].blocks:
        insts = list(blk.instructions)
        new_insts, changed = [], False
        for inst in insts:
            si = getattr(inst, "sync_info", None)
            if si is not None and si.on_wait and len(si.on_wait) > 1:
                waits = list(si.on_wait)
                extra, keep = waits[:-1], waits[-1:]
                for k, w in enumerate(extra):
                    new_insts.append(mb.InstNoOp(
                        name=f"{inst.name}-ws{k}",
                        sync_info=mb.SyncInfo(on_wait=[w], on_update=[]),
                        bass_nofuse=True, engine=inst.engine))
                si.on_wait = keep
                changed = True
            new_insts.append(inst)
        if changed:
            blk.instructions = new_insts


def _build():
    import concourse.bass as bass
    import concourse.tile as tile
    from concourse import mybir as mb

    F32, F32R, BF16 = mb.dt.float32, mb.dt.float32r, mb.dt.bfloat16
    AX = mb.AxisListType.X
    EXP = mb.ActivationFunctionType.Exp
    CPY = mb.ActivationFunctionType.Copy
    IDN = mb.ActivationFunctionType.Identity

    nc = bass.Bass("TRN2", target_bir_lowering=False, debug=False, num_devices=1)

    dq = nc.dram_tensor("query", (SQ, D), F32, kind="ExternalInput").ap()
    dk = nc.dram_tensor("key", (SKV, D), F32, kind="ExternalInput").ap()
    dv = nc.dram_tensor("value", (SKV, D), F32, kind="ExternalInput").ap()
    dW = {t: nc.dram_tensor(f"W{t}", (D, H), F32, kind="ExternalInput").ap()
          for t in "qkv"}
    db = {t: nc.dram_tensor(f"b{t}", (H,), F32, kind="ExternalInput").ap()
          for t in "qkv"}
    dident = nc.dram_tensor("ident", (P, P), F32, kind="ExternalInput").ap()
    dctx = nc.dram_tensor("context", (SQ, H), F32, kind="ExternalOutput").ap()
    dwei = nc.dram_tensor("weights", (SQ, SKV), F32, kind="ExternalOutput").ap()

    with tile.TileContext(nc) as tc:
        with tc.tile_pool(name="const", bufs=1) as const, \
             tc.tile_pool(name="big", bufs=1) as big, \
             tc.tile_pool(name="smal", bufs=2) as smal:

            ident_r = const.tile([P, P], F32R)
            nc.sync.dma_start(out=ident_r, in_=dident.bitcast(F32R))
            # bv broadcast to all partitions (step-0 partition DMA)
            bvb = const.tile([P, H], F32)
            bv_bcast = bass.AP(tensor=db["v"].tensor, offset=db["v"].offset,
                               ap=[[0, P]] + db["v"].ap)
            nc.sync.dma_start(out=bvb, in_=bv_bcast)

            # persistent activations
            qT = [big.tile([P, SQ], F32R, tag=f"qt{h}", name=f"qt{h}") for h in range(HC)]
            kT = [big.tile([P, SKV], F32R, tag=f"kt{h}", name=f"kt{h}") for h in range(HC)]
            v_n = big.tile([P, ST, 512], F32R, tag="vn")

            # ---------------- phase 0/1: weights, biases, inputs ----------
            with tc.tile_pool(name="wpool", bufs=1) as wpool, \
                 tc.tile_pool(name="xstage", bufs=4) as xstage, \
                 tc.tile_pool(name="xt", bufs=1) as xtp, \
                 tc.tile_pool(name="ps1", bufs=2, space="PSUM") as ps1:

                w = {}
                for t in "qkv":
                    w[t] = wpool.tile([P, DC, H], F32R, tag=f"w{t}", name=f"w{t}")
                    nc.sync.dma_start(
                        out=w[t],
                        in_=dW[t].rearrange("(c p) h -> p c h", p=P).bitcast(F32R))
                bT = {}
                for t in "qk":
                    bT[t] = const.tile([P, HC], F32, name=f"bT{t}")
                    nc.sync.dma_start(
                        out=bT[t], in_=db[t].rearrange("(h p) -> p h", p=P))

                xt = [xtp.tile([P, SQ], F32R, tag=f"xt{c}", name=f"xt{c}") for c in range(DC)]

                def load_transpose(dram_t):
                    for i in range(ST):
                        xs = xstage.tile([P, D], F32R, tag="xs")
                        nc.sync.dma_start(
                            out=xs, in_=dram_t[i * P:(i + 1) * P, :].bitcast(F32R))
                        for c in range(DC):
                            pt = ps1.tile([P, P], F32R, tag="tp")
                            nc.tensor.transpose(pt, xs[:, c * P:(c + 1) * P], ident_r)
                            nc.vector.tensor_copy(xt[c][:, i * P:(i + 1) * P], pt)

                def project_T(t, dst):        # dst[h][:, s] = (x@Wt+bt).T
                    for h in range(HC):
                        for cc in range(DC):
                            pp = ps1.tile([P, 512], F32, tag="proj")
                            for c in range(DC):
                                nc.tensor.matmul(
                                    pp, w[t][:, c, h * P:(h + 1) * P],
                                    xt[c][:, cc * 512:(cc + 1) * 512],
                                    start=(c == 0), stop=(c == DC - 1))
                            nc.scalar.activation(
                                dst[h][:, cc * 512:(cc + 1) * 512], pp, IDN,
                                bias=bT[t][:, h:h + 1], scale=1.0)

                load_transpose(dk)
                project_T("k", kT)
                load_transpose(dq)
                project_T("q", qT)
                load_transpose(dv)
                for j in range(ST):           # v natural [kv, H] in bf16
                    pp = ps1.tile([P, 512], F32, tag="proj")
                    for c in range(DC):
                        nc.tensor.matmul(pp, xt[c][:, j * P:(j + 1) * P],
                                         w["v"][:, c, :],
                                         start=(c == 0), stop=(c == DC - 1))
                    nc.vector.tensor_tensor(out=v_n[:, j, :], in0=pp, in1=bvb,
                                            op=mb.AluOpType.add)

            # ---------------- phase 2: attention per q-tile ----------------
            with tc.tile_pool(name="upool", bufs=2) as upool, \
                 tc.tile_pool(name="wout", bufs=2) as wout, \
                 tc.tile_pool(name="wtbs", bufs=8) as wtbs, \
                 tc.tile_pool(name="pssc", bufs=4, space="PSUM") as pssc, \
                 tc.tile_pool(name="pstb", bufs=2, space="PSUM") as pstb, \
                 tc.tile_pool(name="psctx", bufs=2, space="PSUM") as psctx:

                for i in range(ST):
                    qs = slice(i * P, (i + 1) * P)
                    # scores: 4 banks of [128, 512]
                    sc = []
                    nmax4 = smal.tile([P, NB], F32, tag="nmax4")
                    for cc in range(NB):
                        s_ = pssc.tile([P, 512], F32, tag="sc")
                        for h in range(HC):
                            nc.tensor.matmul(s_, qT[h][:, qs],
                                             kT[h][:, cc * 512:(cc + 1) * 512],
                                             start=(h == 0), stop=(h == HC - 1))
                        nc.vector.reduce_max(nmax4[:, cc:cc + 1], s_, axis=AX)
                        sc.append(s_)
                    nm = smal.tile([P, 1], F32, tag="nm")
                    nc.vector.reduce_max(nm, nmax4, axis=AX, negate=True)

                    # U = exp(S - max), unnormalized, in f32r for PE transpose
                    U = upool.tile([P, SKV], F32R, tag="U")
                    sums4 = smal.tile([P, NB], F32, tag="sums4")
                    for cc in range(NB):
                        nc.scalar.activation(U[:, cc * 512:(cc + 1) * 512], sc[cc],
                                             EXP, bias=nm, scale=1.0,
                                             accum_out=sums4[:, cc:cc + 1])
                    ssum = smal.tile([P, 1], F32, tag="ssum")
                    nc.vector.reduce_sum(ssum, sums4, axis=AX)
                    rc = smal.tile([P, 1], F32, tag="rc")
                    nc.vector.reciprocal(rc, ssum)

                    # W fp32 out: normalize on ACT during SBUF->SBUF copy
                    Wt_ = wout.tile([P, SKV], F32, tag="W")
                    nc.scalar.activation(Wt_, U.bitcast(F32), CPY, bias=0.0,
                                         scale=rc)
                    nc.sync.dma_start(out=dwei[qs, :], in_=Wt_)

                    # PE-transpose unnormalized U (f32r), accumulate C' = U @ v
                    pc = psctx.tile([P, 512], F32, tag="ctx")
                    for g in range(4):
                        pt = pstb.tile([P, 512], F32R, tag="wtb")
                        for jj in range(4):
                            j = g * 4 + jj
                            nc.tensor.transpose(pt[:, jj * P:(jj + 1) * P],
                                                U[:, j * P:(j + 1) * P], ident_r)
                        wt_s = wtbs.tile([P, 512], F32R, tag="wtbs")
                        nc.vector.tensor_copy(wt_s, pt)
                        for jj in range(4):
                            j = g * 4 + jj
                            nc.tensor.matmul(pc, wt_s[:, jj * P:(jj + 1) * P],
                                             v_n[:, j, :],
                                             start=(j == 0), stop=(j == ST - 1))
                    # C = C' * recip, normalize during PSUM->SBUF copy on ACT
                    Ct = smal.tile([P, 512], F32, tag="C")
                    nc.scalar.activation(Ct, pc, CPY, bias=0.0, scale=rc)
                    nc.sync.dma_start(out=dctx[qs, :], in_=Ct)

    _patch_multiwaits(nc, mb)
    return nc


def kernel(**inputs):
    from concourse.bass_utils import run_bass_kernel_spmd

    if "nc" not in _CACHE:
        _CACHE["nc"] = _build()
    nc = _CACHE["nc"]

    query = np.asarray(inputs["query"], dtype=np.float32)
    key = np.asarray(inputs["key"], dtype=np.float32)
    value = np.asarray(inputs["value"], dtype=np.float32)
    consts = {
        "Wq": np.asarray(inputs["Wq"], np.float32),
        "Wk": np.asarray(inputs["Wk"], np.float32),
        "Wv": np.asarray(inputs["Wv"], np.float32),
        "bq": np.asarray(inputs["bq"], np.float32),
        "bk": np.asarray(inputs["bk"], np.float32),
        "bv": np.asarray(inputs["bv"], np.float32),
        "ident": np.eye(P, dtype=np.float32),
    }
    in_maps = [dict(consts, query=query[b], key=key[b], value=value[b])
               for b in range(B)]
    res = run_bass_kernel_spmd(nc, in_maps, core_ids=list(range(B)),
                               **_CACHE.get("run_kwargs", {}))
    _CACHE["last_results"] = res
    context = np.stack([res.results[b]["context"] for b in range(B)])
    weights = np.stack([res.results[b]["weights"] for b in range(B)])
    return (context, weights)


# revision 9
# speedup vs baseline: 2.4408x; 1.2347x over previous
"""Fused multi-head-free attention kernel for Trainium2 (Bass/Tile), 8-core SPMD.

Problem: nn_Attention — per batch element b:
    q = query[b] @ Wq + bq          [Sq, H]
    k = key[b]   @ Wk + bk          [Skv, H]
    v = value[b] @ Wv + bv          [Skv, H]
    S = q @ k.T                     [Sq, Skv]
    W = softmax(S, axis=-1)
    C = W @ v                       [Sq, H]
    returns (C, W)

Sharding: pure data-parallel over batch (B=8 == 8 cores), one batch element
per NeuronCore; projection weights replicated. No collectives.

Per-core dataflow (all matmuls on PE, fp32r for the numerically sensitive
path, bf16 for the post-softmax context path):
  1. x in {key, query, value} is DMA'd in 128-row tiles and transposed on PE
     (fp32r transpose via identity) into xT [D, S] layout in SBUF.
  2. qT/kT [H, S] = Wt.T @ xT (fp32r), bias added per-partition during the
     PSUM->SBUF copy.  v is computed in natural [Skv, H] layout (bf16 out).
  3. Per 128-row q-tile: S tile [128, Skv] = qT.T @ kT into 4 PSUM banks,
     row-max on DVE, exp(S - max) on ACT (with per-bank row-sum accumulation),
     normalize -> W (fp32, DMA'd out), and a bf16 copy of W is PE-transposed
     to feed C = W @ v accumulation (bf16), C copied out in fp32.
"""

import numpy as np

B, SQ, SKV, D, H = 8, 2048, 2048, 512, 512
P = 128                 # partitions
ST = SQ // P            # 16 s-tiles
DC = D // P             # 4 contraction chunks
HC = H // P             # 4 h tiles
NB = SKV // 512         # 4 psum banks per score row

_CACHE = {}


def _patch_multiwaits(nc, mb):
    """walrus in this container rejects >1 sync-wait per instruction
    (setupSyncWait: "Too many sync wait commands").  Split extra waits onto
    preceding same-engine NoOps — engine streams are in-order so semantics
    are preserved."""
    for blk in nc.m.functions[0].blocks:
        insts = list(blk.instructions)
        new_insts, changed = [], False
        for inst in insts:
            si = getattr(inst, "sync_info", None)
            if si is not None and si.on_wait and len(si.on_wait) > 1:
                waits = list(si.on_wait)
                extra, keep = waits[:-1], waits[-1:]
                for k, w in enumerate(extra):
                    new_insts.append(mb.InstNoOp(
                        name=f"{inst.name}-ws{k}",
                        sync_info=mb.SyncInfo(on_wait=[w], on_update=[]),
                        bass_nofuse=True, engine=inst.engine))
                si.on_wait = keep
                changed = True
            new_insts.append(inst)
        if changed:
            blk.instructions = new_insts


def _build():
    import concourse.bass as bass
    import concourse.tile as tile
    from concourse import mybir as mb

    F32, F32R, BF16 = mb.dt.float32, mb.dt.float32r, mb.dt.bfloat16
    AX = mb.AxisListType.X
    EXP = mb.ActivationFunctionType.Exp
    CPY = mb.ActivationFunctionType.Copy
    IDN = mb.ActivationFunctionType.Identity

    nc = bass.Bass("TRN2", target_bir_lowering=False, debug=False, num_devices=1)

    dq = nc.dram_tensor("query", (SQ, D), F32, kind="ExternalInput").ap()
    dk = nc.dram_tensor("key", (SKV, D), F32, kind="ExternalInput").ap()
    dv = nc.dram_tensor("value", (SKV, D), F32, kind="ExternalInput").ap()
    dW = {t: nc.dram_tensor(f"W{t}", (D, H), F32, kind="ExternalInput").ap()
          for t in "qkv"}
    db = {t: nc.dram_tensor(f"b{t}", (H,), F32, kind="ExternalInput").ap()
          for t in "qkv"}
    dident = nc.dram_tensor("ident", (P, P), F32, kind="ExternalInput").ap()
    dctx = nc.dram_tensor("context", (SQ, H), F32, kind="ExternalOutput").ap()
    dwei = nc.dram_tensor("weights", (SQ, SKV), F32, kind="ExternalOutput").ap()

    with tile.TileContext(nc) as tc:
        with tc.tile_pool(name="const", bufs=1) as const, \
             tc.tile_pool(name="big", bufs=1) as big, \
             tc.tile_pool(name="smal", bufs=2) as smal:

            ident_r = const.tile([P, P], F32R)
            nc.sync.dma_start(out=ident_r, in_=dident.bitcast(F32R))
            # bv broadcast to all partitions (step-0 partition DMA)
            bvb = const.tile([P, H], F32)
            bv_bcast = bass.AP(tensor=db["v"].tensor, offset=db["v"].offset,
                               ap=[[0, P]] + db["v"].ap)
            nc.sync.dma_start(out=bvb, in_=bv_bcast)

            # persistent activations
            qT = [big.tile([P, SQ], F32R, tag=f"qt{h}", name=f"qt{h}") for h in range(HC)]
            kT = [big.tile([P, SKV], F32R, tag=f"kt{h}", name=f"kt{h}") for h in range(HC)]
            v_n = big.tile([P, ST, 512], F32R, tag="vn")

            # ---------------- phase 0/1: weights, biases, inputs ----------
            with tc.tile_pool(name="wpool", bufs=1) as wpool, \
                 tc.tile_pool(name="xstage", bufs=4) as xstage, \
                 tc.tile_pool(name="xt", bufs=1) as xtp, \
                 tc.tile_pool(name="ps1", bufs=2, space="PSUM") as ps1:

                w = {}
                for t in "qkv":
                    w[t] = wpool.tile([P, DC, H], F32R, tag=f"w{t}", name=f"w{t}")
                    nc.sync.dma_start(
                        out=w[t],
                        in_=dW[t].rearrange("(c p) h -> p c h", p=P).bitcast(F32R))
                bT = {}
                for t in "qk":
                    bT[t] = const.tile([P, HC], F32, name=f"bT{t}")
                    nc.sync.dma_start(
                        out=bT[t], in_=db[t].rearrange("(h p) -> p h", p=P))

                xt3 = xtp.tile([P, DC, SQ], F32R, tag="xt3", name="xt3")
                xt = [xt3[:, c, :] for c in range(DC)]

                def load_transpose(dram_t):
                    # 4 transposed 128x128 blocks land side by side in one
                    # psum bank; one wide strided DVE copy moves all 4 to xt3.
                    xt_v = xt3.rearrange("p c (i q) -> p c i q", q=P)
                    for i in range(ST):
                        xs = xstage.tile([P, D], F32R, tag="xs")
                        nc.sync.dma_start(
                            out=xs, in_=dram_t[i * P:(i + 1) * P, :].bitcast(F32R))
                        pt = ps1.tile([P, DC, P], F32R, tag="tp")
                        for c in range(DC):
                            nc.tensor.transpose(pt[:, c, :], xs[:, c * P:(c + 1) * P],
                                                ident_r)
                        nc.vector.tensor_copy(xt_v[:, :, i, :], pt)

                def project_T(t, dst):        # dst[h][:, s] = (x@Wt+bt).T
                    for h in range(HC):
                        for cc in range(DC):
                            pp = ps1.tile([P, 512], F32, tag="proj")
                            for c in range(DC):
                                nc.tensor.matmul(
                                    pp, w[t][:, c, h * P:(h + 1) * P],
                                    xt[c][:, cc * 512:(cc + 1) * 512],
                                    start=(c == 0), stop=(c == DC - 1))
                            nc.scalar.activation(
                                dst[h][:, cc * 512:(cc + 1) * 512], pp, IDN,
                                bias=bT[t][:, h:h + 1], scale=1.0)

                load_transpose(dk)
                project_T("k", kT)
                load_transpose(dq)
                project_T("q", qT)
                load_transpose(dv)
                for j in range(ST):           # v natural [kv, H] in bf16
                    pp = ps1.tile([P, 512], F32, tag="proj")
                    for c in range(DC):
                        nc.tensor.matmul(pp, xt[c][:, j * P:(j + 1) * P],
                                         w["v"][:, c, :],
                                         start=(c == 0), stop=(c == DC - 1))
                    nc.vector.tensor_tensor(out=v_n[:, j, :], in0=pp, in1=bvb,
                                            op=mb.AluOpType.add)

            # ---------------- phase 2: attention, software-pipelined -------
            # Emission order interleaves tile i+1's score matmuls with tile
            # i's exp/transpose/context tail so the PE never drains while ACT
            # computes exp.  PSUM: scores bufs=5 + transpose 2 + context 1 = 8.
            with tc.tile_pool(name="upool", bufs=2) as upool, \
                 tc.tile_pool(name="wout", bufs=2) as wout, \
                 tc.tile_pool(name="wtbs", bufs=8) as wtbs, \
                 tc.tile_pool(name="pssc", bufs=5, space="PSUM") as pssc, \
                 tc.tile_pool(name="pstb", bufs=2, space="PSUM") as pstb, \
                 tc.tile_pool(name="psctx", bufs=1, space="PSUM") as psctx:

                def emit_scores(i):
                    qs = slice(i * P, (i + 1) * P)
                    sc = []
                    nmax4 = smal.tile([P, NB], F32, tag="nmax4")
                    for cc in range(NB):
                        s_ = pssc.tile([P, 512], F32, tag="sc")
                        for h in range(HC):
                            nc.tensor.matmul(s_, qT[h][:, qs],
                                             kT[h][:, cc * 512:(cc + 1) * 512],
                                             start=(h == 0), stop=(h == HC - 1))
                        nc.vector.reduce_max(nmax4[:, cc:cc + 1], s_, axis=AX)
                        sc.append(s_)
                    return sc, nmax4

                def emit_tail(i, sc, nmax4):
                    qs = slice(i * P, (i + 1) * P)
                    nm = smal.tile([P, 1], F32, tag="nm")
                    nc.vector.reduce_max(nm, nmax4, axis=AX, negate=True)

                    U = upool.tile([P, SKV], F32R, tag="U")
                    sums4 = smal.tile([P, NB], F32, tag="sums4")
                    for cc in range(NB):
                        nc.scalar.activation(U[:, cc * 512:(cc + 1) * 512], sc[cc],
                                             EXP, bias=nm, scale=1.0,
                                             accum_out=sums4[:, cc:cc + 1])
                    ssum = smal.tile([P, 1], F32, tag="ssum")
                    nc.vector.reduce_sum(ssum, sums4, axis=AX)
                    rc = smal.tile([P, 1], F32, tag="rc")
                    nc.vector.reciprocal(rc, ssum)

                    # W fp32 out: normalize on ACT during SBUF->SBUF copy
                    Wt_ = wout.tile([P, SKV], F32, tag="W")
                    nc.scalar.activation(Wt_, U.bitcast(F32), CPY, bias=0.0,
                                         scale=rc)
                    nc.sync.dma_start(out=dwei[qs, :], in_=Wt_)

                    # PE-transpose unnormalized U (f32r), accumulate C' = U @ v
                    pc = psctx.tile([P, 512], F32, tag="ctx")
                    for g in range(4):
                        pt = pstb.tile([P, 512], F32R, tag="wtb")
                        for jj in range(4):
                            j = g * 4 + jj
                            nc.tensor.transpose(pt[:, jj * P:(jj + 1) * P],
                                                U[:, j * P:(j + 1) * P], ident_r)
                        wt_s = wtbs.tile([P, 512], F32R, tag="wtbs")
                        nc.vector.tensor_copy(wt_s, pt)
                        for jj in range(4):
                            j = g * 4 + jj
                            nc.tensor.matmul(pc, wt_s[:, jj * P:(jj + 1) * P],
                                             v_n[:, j, :],
                                             start=(j == 0), stop=(j == ST - 1))
                    # C = C' * recip, normalized during PSUM->SBUF copy on ACT
                    Ct = smal.tile([P, 512], F32, tag="C")
                    nc.scalar.activation(Ct, pc, CPY, bias=0.0, scale=rc)
                    nc.sync.dma_start(out=dctx[qs, :], in_=Ct)

                pending = None
                for i in range(ST):
                    sc_nm = emit_scores(i)
                    if pending is not None:
                        emit_tail(*pending)
                    pending = (i,) + sc_nm
                emit_tail(*pending)

    _patch_multiwaits(nc, mb)
    return nc


def kernel(**inputs):
    from concourse.bass_utils import run_bass_kernel_spmd

    if "nc" not in _CACHE:
        _CACHE["nc"] = _build()
    nc = _CACHE["nc"]

    query = np.asarray(inputs["query"], dtype=np.float32)
    key = np.asarray(inputs["key"], dtype=np.float32)
    value = np.asarray(inputs["value"], dtype=np.float32)
    consts = {
        "Wq": np.asarray(inputs["Wq"], np.float32),
        "Wk": np.asarray(inputs["Wk"], np.float32),
        "Wv": np.asarray(inputs["Wv"], np.float32),
        "bq": np.asarray(inputs["bq"], np.float32),
        "bk": np.asarray(inputs["bk"], np.float32),
        "bv": np.asarray(inputs["bv"], np.float32),
        "ident": np.eye(P, dtype=np.float32),
    }
    in_maps = [dict(consts, query=query[b], key=key[b], value=value[b])
               for b in range(B)]
    res = run_bass_kernel_spmd(nc, in_maps, core_ids=list(range(B)),
                               **_CACHE.get("run_kwargs", {}))
    _CACHE["last_results"] = res
    context = np.stack([res.results[b]["context"] for b in range(B)])
    weights = np.stack([res.results[b]["weights"] for b in range(B)])
    return (context, weights)
